# revision 13
# baseline (speedup 1.0000x reference)
"""Fused transformer block (LN -> QKV+RoPE -> attention -> out_proj) on 8
Trainium2 NeuronCores.

Sharding: batch (2-way) x heads (4-way) = 8 cores. Core c handles batch
b = c // 4 and the 4 heads starting at 4*(c%4). Each core produces the
out_proj partial sum over its 256 dh-dims; the host sums 4 partials per
batch and adds b_out.

Device math, per core (matmul inputs fp16, PSUM accum fp32):
- x passed transposed AND pre-cast to fp16 on host: xT [D, S].
- LN stats via TensorE ones-matmuls (sum_d x, sum_d x^2 over partitions),
  one fused k-loop so PE chases the xT DMA.
- ln_g folded into the weights on host (weights fp16; wsum computed from
  the fp16-rounded weights so the mu-folding matches the matmul).
- Mean-centering folded into the matmul epilogue ON PE: a K=1 matmul
  with lhsT = -wsum (f16) and rhs = mu (f16) accumulates -mu*wsum into
  the same PSUM group, so u = Z - mu*wsum comes out of PSUM directly.
- RoPE in [e, s] layout; u copied PSUM->SBUF fp16, rotate-half swap via
  cheap fp16 partition-sliced SBUF->SBUF DMAs, then all-fp16 DVE muls
  with host-precomputed cos/sin (sin carries the rotate-half signs).
  Per-position rstd: applied to q explicitly (fp16 broadcast), folded
  into exp's per-partition scale on the k side; v applies rstd in its
  PSUM epilogue.
- attention per (head, i-half): scores^T[j,i] K=64 one N=1024 matmul;
  exp on ScalarE (scale = rstd_k[j]/8) -> fp16 probs; o^T accumulated
  over j with lhsT = [v | 1] (M=65, row 64 = softmax denominators).
  Software-pipelined emission (scores_{j+1} before AV_j) keeps the PE
  FIFO from head-blocking on the exp; po double-buffered (i-halved) so
  the denominator/reciprocal chain never stalls the next accumulation.
- out_proj partial from o^T tiles; PSUM->SBUF copies on ScalarE (idle
  after the last exp); DMA to HBM fp16; host upcasts, reduces, + b_out.
"""
import sys
sys.path.insert(0, "/opt/trn_rl_repo")
import numpy as np

B, S, D = 2, 2048, 1024
HEADS, HDIM = 16, 64
HALF = HDIM // 2
ROPE_THETA = 10000.0
N_CORES = 8
HPC = HEADS // 4            # heads per core = 4
EC = HPC * HDIM             # per-core q (or k, or v) width = 256
P = 128
NK = D // P                 # 8 d-tiles
NS = S // P                 # 16 s-tiles
SH = S // 2                 # i-half width = 1024
VW = HDIM + 1               # v block width incl. ones column = 65

_cache = {}


def _build():
    import contextlib
    import concourse.bass as bass
    import concourse.bacc as bacc
    import concourse.tile as tile
    from concourse import mybir
    fp32 = mybir.dt.float32
    f16 = mybir.dt.float16
    OP = mybir.AluOpType
    AF = mybir.ActivationFunctionType

    nc = bacc.Bacc("TRN2", target_bir_lowering=False, debug=False,
                   enable_asserts=True, num_devices=N_CORES)

    xT = nc.dram_tensor("xT", [D, S], f16, kind="ExternalInput").ap()
    wqkT = nc.dram_tensor("wqkT", [D, 2 * EC], f16, kind="ExternalInput").ap()
    wvT = nc.dram_tensor("wvT", [D, EC], f16, kind="ExternalInput").ap()
    woT = nc.dram_tensor("woT", [EC, D], f16, kind="ExternalInput").ap()
    nws_qk = nc.dram_tensor("nws_qk", [1, 2 * EC], f16,
                            kind="ExternalInput").ap()
    nws_v = nc.dram_tensor("nws_v", [1, EC], f16, kind="ExternalInput").ap()
    cosf = nc.dram_tensor("cosf", [P, S], f16, kind="ExternalInput").ap()
    sinsg = nc.dram_tensor("sinsg", [P, S], f16, kind="ExternalInput").ap()
    out = nc.dram_tensor("out", [S, D], f16, kind="ExternalOutput").ap()

    with tile.TileContext(nc) as tc, contextlib.ExitStack() as ctx:
        singles = ctx.enter_context(tc.tile_pool(name="singles", bufs=1))
        dram_scr = ctx.enter_context(
            tc.tile_pool(name="dram_scr", bufs=1, space="DRAM"))
        qk_sb = singles.tile([P, 4, S], f16)              # 16KB/part
        v_sb = singles.tile([P, NS, HPC * VW], f16)       # 8.1KB/part
        nc.gpsimd.memset(v_sb[:], 1.0)
        rstdT = singles.tile([P, NS], fp32)
        rstdT8 = singles.tile([P, NS], fp32)
        onep = singles.tile([P, 2], fp32)
        nc.vector.memset(onep[:], 1.0)
        nc.vector.memset(onep[0:1, 1:2], 1e-5)
        eps_sb = onep[0:1, 1:2]
        ones16 = singles.tile([P, 1], f16)
        nc.vector.memset(ones16[:], 1.0)
        ones_sb = ones16[:, 0:1]
        mu16 = singles.tile([1, S], f16)
        rstd16 = singles.tile([1, S], f16)
        rstd16_b = singles.tile([P, S], f16)
        nws_sb = singles.tile([1, 2 * EC], f16)
        nwsv_sb = singles.tile([1, EC], f16)
        cos_sb = singles.tile([P, S], f16)
        sin_sb = singles.tile([P, S], f16)
        wo_sb = singles.tile([P, 2, D], f16)
        oT_sb = singles.tile([P, 2, S], f16)       # o^T (4 heads x 64 rows)

        with tc.tile_pool(name="ph1a", bufs=1) as ph1a:
            xT_sb = ph1a.tile([P, NK, S], f16)            # 32KB/part
            wqk_sb = ph1a.tile([P, NK, 4 * P], f16)       # 8KB/part
            wv_sb = ph1a.tile([P, NK, EC], f16)           # 4KB/part
            xT_r = xT.rearrange("(k p) s -> p k s", p=P)
            wqk_r = wqkT.rearrange("(k p) e -> p k e", p=P)
            wv_r = wvT.rearrange("(k p) e -> p k e", p=P)
            wo_r = woT.rearrange("(k p) e -> p k e", p=P)
            # prefetch everything on the sync queue; x k0 first so LN
            # stats start immediately, weights next so no Ldweights stall.
            nc.sync.dma_start(out=xT_sb[:, 0, :], in_=xT_r[:, 0, :])
            nc.sync.dma_start(out=wqk_sb[:], in_=wqk_r[:])
            for k in range(1, NK):
                nc.sync.dma_start(out=xT_sb[:, k, :], in_=xT_r[:, k, :])
            nc.sync.dma_start(out=wv_sb[:], in_=wv_r[:])
            nc.sync.dma_start(out=wo_sb[:], in_=wo_r[:])
            nc.sync.dma_start(out=cos_sb[:], in_=cosf[:])
            nc.sync.dma_start(out=sin_sb[:], in_=sinsg[:])
            nc.sync.dma_start(out=nws_sb[:], in_=nws_qk[:])
            nc.sync.dma_start(out=nwsv_sb[:], in_=nws_v[:])

            # ---------------- phase 0: LN stats ----------------
            with tc.tile_pool(name="p0ps_a", bufs=1, space="PSUM") as p0ps_a, \
                 tc.tile_pool(name="p0ps_b", bufs=1, space="PSUM") as p0ps_b, \
                 tc.tile_pool(name="p0scr", bufs=1) as p0scr, \
                 tc.tile_pool(name="p0tmp", bufs=3) as p0tmp:
                rstd_sb = p0scr.tile([1, S], fp32)
                pss = [p0ps_a.tile([1, 512], fp32, tag=f"pss{c}",
                                   name=f"pss{c}") for c in range(4)]
                psq = [p0ps_b.tile([1, 512], fp32, tag=f"psq{c}",
                                   name=f"psq{c}") for c in range(4)]
                for k in range(NK):
                    for c in range(4):
                        nc.tensor.matmul(pss[c][:], ones_sb,
                                         xT_sb[:, k, c * 512:(c + 1) * 512],
                                         start=(k == 0), stop=(k == NK - 1),
                                         skip_group_check=True)
                    for h2 in range(2):
                        xsq = p0tmp.tile([P, S // 2], f16, tag="xsq")
                        nc.vector.tensor_mul(
                            xsq[:], xT_sb[:, k, h2 * 1024:(h2 + 1) * 1024],
                            xT_sb[:, k, h2 * 1024:(h2 + 1) * 1024])
                        for c in range(2):
                            ci = h2 * 2 + c
                            nc.tensor.matmul(psq[ci][:], ones_sb,
                                             xsq[:, c * 512:(c + 1) * 512],
                                             start=(k == 0), stop=(k == NK - 1),
                                             skip_group_check=True)
                for c in range(4):   # mu (f16) straight from PSUM on Act
                    nc.scalar.mul(out=mu16[:, c * 512:(c + 1) * 512],
                                  in_=pss[c][:], mul=1.0 / D)
                # var = ssq/D - mu^2 ; rstd = 1/sqrt(var + eps)
                nc.vector.tensor_mul(rstd_sb[:], mu16[:], mu16[:])
                for c in range(4):
                    nc.vector.scalar_tensor_tensor(
                        out=rstd_sb[:, c * 512:(c + 1) * 512],
                        in0=psq[c][:], scalar=1.0 / D,
                        in1=rstd_sb[:, c * 512:(c + 1) * 512],
                        op0=OP.mult, op1=OP.subtract)
                nc.scalar.activation(rstd_sb[:], rstd_sb[:], AF.Sqrt,
                                     bias=eps_sb)
                nc.vector.reciprocal(out=rstd_sb[:], in_=rstd_sb[:])
                nc.scalar.mul(out=rstd16[:], in_=rstd_sb[:], mul=1.0)

                # SBUF->SBUF partition-broadcast is illegal; bounce via DRAM.
                rstd_d = dram_scr.tile([1, S], fp32)
                rstd16_d = dram_scr.tile([1, S], f16)
                nc.sync.dma_start(out=rstd_d[:], in_=rstd_sb[:])
                nc.sync.dma_start(out=rstd16_d[:], in_=rstd16[:])
                _rd, _r6 = rstd_d[:], rstd16_d[:]
                nc.sync.dma_start(
                    out=rstd16_b[:],
                    in_=bass.AP(tensor=_r6.tensor, offset=_r6.offset,
                                ap=[[0, P], [1, S]]))
                # transposed per-s-tile scalars: [p, t] = vec[t*128 + p]
                nc.sync.dma_start(
                    out=rstdT[:],
                    in_=bass.AP(tensor=_rd.tensor, offset=_rd.offset,
                                ap=[[1, P], [P, NS]]))
                nc.vector.tensor_scalar_mul(rstdT8[:], rstdT[:],
                                            float(HDIM) ** -0.5)

            # ---------------- phase 1a: Q/K matmuls + RoPE ----------------
            # k-side (e=2,3) first: it doesn't need rstd16_b yet.
            with tc.tile_pool(name="p1psum", bufs=3, space="PSUM") as p1psum, \
                 tc.tile_pool(name="p1tmp", bufs=3) as p1tmp:
                for e in (2, 3, 0, 1):
                    for sh in range(2):
                        s0 = sh * SH
                        zq = p1psum.tile([P, SH], fp32, tag="zqk")
                        for c in range(2):   # fp32 PSUM: matmul N <= 512
                            c0 = c * 512
                            for k in range(NK):
                                nc.tensor.matmul(
                                    zq[:, c0:c0 + 512],
                                    wqk_sb[:, k, e * P:(e + 1) * P],
                                    xT_sb[:, k, s0 + c0:s0 + c0 + 512],
                                    start=(k == 0), stop=False,
                                    skip_group_check=True)
                            # u = Z - mu*wsum via K=1 matmul (mu moving)
                            nc.tensor.matmul(
                                zq[:, c0:c0 + 512],
                                nws_sb[:, e * P:(e + 1) * P],
                                mu16[:, s0 + c0:s0 + c0 + 512],
                                start=False, stop=True,
                                skip_group_check=True)
                        u = p1tmp.tile([P, SH], f16, tag="u")
                        nc.scalar.mul(out=u[:], in_=zq[:], mul=1.0)
                        # rotate-half swap (per 64-row head group)
                        usw = p1tmp.tile([P, SH], f16, tag="usw")
                        for g in range(2):
                            b0 = g * HDIM
                            nc.sync.dma_start(out=usw[b0:b0 + HALF, :],
                                              in_=u[b0 + HALF:b0 + HDIM, :])
                            nc.sync.dma_start(out=usw[b0 + HALF:b0 + HDIM, :],
                                              in_=u[b0:b0 + HALF, :])
                        # rot = u*cos + usw*sin_signed (all-fp16 SBUF DVE)
                        nc.vector.tensor_mul(u[:], u[:], cos_sb[:, s0:s0 + SH])
                        nc.vector.tensor_mul(usw[:], usw[:],
                                             sin_sb[:, s0:s0 + SH])
                        if e < 2:   # q side: multiply by rstd as well
                            nc.vector.tensor_add(u[:], u[:], usw[:])
                            nc.vector.tensor_mul(qk_sb[:, e, s0:s0 + SH],
                                                 u[:],
                                                 rstd16_b[:, s0:s0 + SH])
                        else:
                            nc.vector.tensor_add(qk_sb[:, e, s0:s0 + SH],
                                                 u[:], usw[:])

            # ---------------- phase 1b: V (natural layout) ----------------
            with tc.tile_pool(name="p1vps", bufs=3, space="PSUM") as p1vps:
                for t in range(NS):
                    zv = p1vps.tile([P, EC], fp32, tag="zv")
                    for k in range(NK):
                        nc.tensor.matmul(zv[:], xT_sb[:, k, t * P:(t + 1) * P],
                                         wv_sb[:, k, :],
                                         start=(k == 0), stop=False)
                    nc.tensor.matmul(zv[:], mu16[:, t * P:(t + 1) * P],
                                     nwsv_sb[:], start=False, stop=True)
                    # v = rstd_s * (Zv - mu*wsum)
                    nc.vector.tensor_scalar_mul(
                        v_sb[:, t, :].rearrange("p (h w) -> p h w",
                                                h=HPC)[:, :, 0:HDIM],
                        zv[:].rearrange("p (h d) -> p h d", h=HPC),
                        rstdT[:, t:t + 1])

        # ---------------- phase 2: attention ----------------
        with tc.tile_pool(name="ps_s", bufs=2, space="PSUM") as ps_s, \
             tc.tile_pool(name="ps_o", bufs=2, space="PSUM") as ps_o, \
             tc.tile_pool(name="p2tmp", bufs=3) as p2tmp, \
             tc.tile_pool(name="p2rec", bufs=2) as p2rec, \
             tc.tile_pool(name="p2recd", bufs=2, space="DRAM") as p2recd:
            for h in range(HPC):
                et = h // 2
                ep = (h % 2) * HDIM
                for ih in range(2):
                    i0 = ih * SH
                    po = ps_o.tile([VW, SH], fp32, tag="po")
                    p_prev = None
                    for j in range(NS):
                        pscore = ps_s.tile([P, SH], fp32, tag="ps")
                        for c in range(2):   # fp32 PSUM: matmul N <= 512
                            c0 = c * 512
                            nc.tensor.matmul(
                                pscore[:, c0:c0 + 512],
                                qk_sb[ep:ep + HDIM, 2 + et, j * P:(j + 1) * P],
                                qk_sb[ep:ep + HDIM, et,
                                      i0 + c0:i0 + c0 + 512],
                                start=True, stop=True,
                                skip_group_check=True)
                        p_sb = p2tmp.tile([P, SH], f16, tag="p")
                        nc.scalar.activation(p_sb[:], pscore[:], AF.Exp,
                                             scale=rstdT8[:, j:j + 1])
                        if p_prev is not None:
                            for c in range(2):
                                nc.tensor.matmul(po[:, c * 512:(c + 1) * 512],
                                                 v_sb[:, j - 1,
                                                      h * VW:(h + 1) * VW],
                                                 p_prev[:, c * 512:(c + 1) * 512],
                                                 start=(j == 1), stop=False,
                                                 skip_group_check=True)
                        p_prev = p_sb
                    for c in range(2):
                        nc.tensor.matmul(po[:, c * 512:(c + 1) * 512],
                                         v_sb[:, NS - 1, h * VW:(h + 1) * VW],
                                         p_prev[:, c * 512:(c + 1) * 512],
                                         start=False, stop=True,
                                         skip_group_check=True)
                    rec = p2rec.tile([1, SH], fp32, tag="rec")
                    nc.vector.reciprocal(out=rec[:], in_=po[HDIM:HDIM + 1, :])
                    rec_d = p2recd.tile([1, SH], fp32, tag="recd",
                                        name="rec_d")
                    nc.sync.dma_start(out=rec_d[:], in_=rec[:])
                    recb = p2rec.tile([HDIM, SH], fp32, tag="recb")
                    _rc = rec_d[:]
                    nc.sync.dma_start(
                        out=recb[:],
                        in_=bass.AP(tensor=_rc.tensor, offset=_rc.offset,
                                    ap=[[0, HDIM], [1, SH]]))
                    nc.vector.tensor_mul(oT_sb[ep:ep + HDIM, et, i0:i0 + SH],
                                         po[0:HDIM, :], recb[:])

        # ---------------- phase 3: out_proj partial ----------------
        with tc.tile_pool(name="p3psum", bufs=2, space="PSUM") as p3psum, \
             tc.tile_pool(name="p3tmp", bufs=3) as p3tmp:
            for t in range(NS):
                pout = p3psum.tile([P, D], fp32, tag="pout")
                for c in range(2):   # fp32 PSUM: matmul N <= 512
                    c0 = c * 512
                    for k in range(2):
                        nc.tensor.matmul(pout[:, c0:c0 + 512],
                                         oT_sb[:, k, t * P:(t + 1) * P],
                                         wo_sb[:, k, c0:c0 + 512],
                                         start=(k == 0), stop=(k == 1),
                                         skip_group_check=True)
                ot = p3tmp.tile([P, D], f16, tag="ot")
                nc.scalar.mul(out=ot[:], in_=pout[:], mul=1.0)
                nc.sync.dma_start(out=out[t * P:(t + 1) * P, :], in_=ot[:])

    nc.compile()
    return nc


def _host_inputs(x, ln_g, ln_b, w_qkv, w_out):
    import ml_dtypes
    f16 = np.float16
    wq = w_qkv[0:D] * ln_g[None, :]
    wk = w_qkv[D:2 * D] * ln_g[None, :]
    wv = w_qkv[2 * D:3 * D] * ln_g[None, :]
    if np.abs(w_qkv.astype(np.float32) @ ln_b.astype(np.float32)).max() != 0.0:
        raise NotImplementedError("nonzero ln_b not supported")
    inv = 1.0 / (ROPE_THETA ** (np.arange(0, HALF, dtype=np.float32) / HALF))
    fr = np.arange(S, dtype=np.float32)[:, None] * inv[None, :]
    cos = np.cos(fr).T.astype(np.float32)          # [32, S]
    sin = np.sin(fr).T.astype(np.float32)
    # row layout per 64-group: [lo(32); hi(32)]; cos same both halves.
    cosf = np.tile(cos, (4, 1)).astype(f16)       # [128, S]
    # rot_lo = lo*c - hi*s ; rot_hi = hi*c + lo*s. usw = [hi; lo], so the
    # sin multiplier rows are [-s (for lo out); +s (for hi out)].
    sinsg = np.tile(np.concatenate([-sin, sin], 0), (2, 1)).astype(f16)
    ins = []
    for core in range(N_CORES):
        b = core // 4
        h0 = (core % 4) * HPC
        sl = slice(h0 * HDIM, (h0 + HPC) * HDIM)
        wq_c, wk_c, wv_c = wq[sl], wk[sl], wv[sl]
        qk16 = np.concatenate([wq_c, wk_c], 0).astype(f16)
        wv16 = wv_c.astype(f16)
        # wsum from the fp16-rounded weights so mu-folding matches; negated
        # because the K=1 matmul ACCUMULATES -mu*wsum into Z.
        nws_qk = (-qk16.astype(np.float32).sum(1)).astype(f16)[None, :]
        nws_v = (-wv16.astype(np.float32).sum(1)).astype(f16)[None, :]
        ins.append({
            "xT": np.ascontiguousarray(x[b].T.astype(f16)),
            "wqkT": np.ascontiguousarray(qk16.T),
            "wvT": np.ascontiguousarray(wv16.T),
            "woT": np.ascontiguousarray(w_out[:, sl].T.astype(f16)),
            "nws_qk": nws_qk, "nws_v": nws_v,
            "cosf": cosf, "sinsg": sinsg,
        })
    return ins


def kernel(x, ln_g, ln_b, w_qkv, w_out, b_out):
    from concourse import bass_utils
    x = np.asarray(x, np.float32)
    ln_g = np.asarray(ln_g, np.float32)
    ln_b = np.asarray(ln_b, np.float32)
    w_qkv = np.asarray(w_qkv, np.float32)
    w_out = np.asarray(w_out, np.float32)
    b_out = np.asarray(b_out, np.float32)
    if "nc" not in _cache:
        _cache["nc"] = _build()
    ins = _host_inputs(x, ln_g, ln_b, w_qkv, w_out)
    res = bass_utils.run_bass_kernel_spmd(_cache["nc"], ins,
                                          core_ids=list(range(N_CORES)))
    _cache["last_results"] = res
    out = np.zeros((B, S, D), np.float32)
    for core in range(N_CORES):
        out[core // 4] += res.results[core]["out"]
    out += b_out[None, None, :]
    return out


# revision 15
# speedup vs baseline: 3.5276x; 3.5276x over previous
"""Fused transformer block (LN -> QKV+RoPE -> attention -> out_proj) on 8
Trainium2 NeuronCores.

Sharding: batch (2-way) x heads (4-way) = 8 cores. Core c handles batch
b = c // 4 and the 4 heads starting at 4*(c%4). Each core produces the
out_proj partial sum over its 256 dh-dims; the host sums 4 partials per
batch and adds b_out.

Device math, per core (matmul inputs fp16, PSUM accum fp32):
- x passed transposed AND pre-cast to fp16 on host: xT [D, S].
- LN stats via TensorE ones-matmuls; sum pass then square pass so the
  sum PSUM banks free early for the QKV matmuls.
- ln_g folded into the weights on host (weights fp16; wsum computed from
  the fp16-rounded weights). Mean-centering folded into the matmul
  epilogue ON PE: a K=1 matmul with lhsT = -wsum (f16) and rhs = mu
  (f16) accumulates -mu*wsum into the same PSUM group, so
  u = Z - mu*wsum comes straight out of PSUM.
- V computed FIRST (needs no RoPE), then q0/k0 RoPE groups, so the
  first attention block's scores+exp start ~80us earlier than a strict
  phase ordering: exp for (h0,ih0/ih1) is emitted interleaved with the
  remaining q1/k1 RoPE matmuls (worklist), keeping ScalarE's exp stream
  continuous from ~t=50us — exp is the critical 133us resource.
- RoPE in [e, s] layout; u copied PSUM->SBUF fp16 on DVE, rotate-half
  swap via cheap fp16 partition-sliced SBUF->SBUF DMAs, then all-fp16
  DVE muls with host cos/sin (sin carries the rotate-half signs).
  Per-position rstd: on q explicitly, in exp's scale for k, in the
  PSUM epilogue for v.
- attention pipelined one FULL block (h,ih) deep: block b's AV matmuls
  interleave with block b+2's scores/exp (b+1's were emitted in the
  previous iteration); ~32 prob tiles held in SBUF. o^T accumulated
  over j with lhsT = [v | 1] (M=65, row 64 = softmax denominators).
- out_proj partial from o^T; PSUM->SBUF copies on ScalarE (idle after
  the last exp); DMA to HBM fp16; host upcasts, reduces, adds b_out.
"""
import sys
sys.path.insert(0, "/opt/trn_rl_repo")
import numpy as np

B, S, D = 2, 2048, 1024
HEADS, HDIM = 16, 64
HALF = HDIM // 2
ROPE_THETA = 10000.0
N_CORES = 8
HPC = HEADS // 4            # heads per core = 4
EC = HPC * HDIM             # per-core q (or k, or v) width = 256
P = 128
NK = D // P                 # 8 d-tiles
NS = S // P                 # 16 s-tiles
SH = S // 2                 # i-half width = 1024
VW = HDIM + 1               # v block width incl. ones column = 65

_cache = {}


def _build():
    import contextlib
    import concourse.bass as bass
    import concourse.bacc as bacc
    import concourse.tile as tile
    from concourse import mybir
    fp32 = mybir.dt.float32
    f16 = mybir.dt.float16
    OP = mybir.AluOpType
    AF = mybir.ActivationFunctionType

    nc = bacc.Bacc("TRN2", target_bir_lowering=False, debug=False,
                   enable_asserts=True, num_devices=N_CORES)

    xT = nc.dram_tensor("xT", [D, S], f16, kind="ExternalInput").ap()
    wqkT = nc.dram_tensor("wqkT", [D, 2 * EC], f16, kind="ExternalInput").ap()
    wvT = nc.dram_tensor("wvT", [D, EC], f16, kind="ExternalInput").ap()
    woT = nc.dram_tensor("woT", [EC, D], f16, kind="ExternalInput").ap()
    nws_qk = nc.dram_tensor("nws_qk", [1, 2 * EC], f16,
                            kind="ExternalInput").ap()
    nws_v = nc.dram_tensor("nws_v", [1, EC], f16, kind="ExternalInput").ap()
    cosf = nc.dram_tensor("cosf", [P, S], f16, kind="ExternalInput").ap()
    sinsg = nc.dram_tensor("sinsg", [P, S], f16, kind="ExternalInput").ap()
    out = nc.dram_tensor("out", [S, D], f16, kind="ExternalOutput").ap()

    with tile.TileContext(nc) as tc, contextlib.ExitStack() as ctx:
        singles = ctx.enter_context(tc.tile_pool(name="singles", bufs=1))
        dram_scr = ctx.enter_context(
            tc.tile_pool(name="dram_scr", bufs=1, space="DRAM"))
        qk_sb = singles.tile([P, 4, S], f16)              # 16KB/part
        v_sb = singles.tile([P, NS, HPC * VW], f16)       # 8.1KB/part
        nc.gpsimd.memset(v_sb[:], 1.0)
        rstdT = singles.tile([P, NS], fp32)
        rstdT8 = singles.tile([P, NS], fp32)
        onep = singles.tile([P, 2], fp32)
        nc.vector.memset(onep[:], 1.0)
        nc.vector.memset(onep[0:1, 1:2], 1e-5)
        eps_sb = onep[0:1, 1:2]
        ones16 = singles.tile([P, 1], f16)
        nc.vector.memset(ones16[:], 1.0)
        ones_sb = ones16[:, 0:1]
        mu16 = singles.tile([1, S], f16)
        rstd16 = singles.tile([1, S], f16)
        rstd16_b = singles.tile([P, S], f16)
        nws_sb = singles.tile([1, 2 * EC], f16)
        nwsv_sb = singles.tile([1, EC], f16)
        cos_sb = singles.tile([P, S], f16)
        sin_sb = singles.tile([P, S], f16)
        wo_sb = singles.tile([P, 2, D], f16)
        oT_sb = singles.tile([P, 2, S], f16)       # o^T (4 heads x 64 rows)

        with tc.tile_pool(name="ph1a", bufs=1) as ph1a:
            xT_sb = ph1a.tile([P, NK, S], f16)            # 32KB/part
            wqk_sb = ph1a.tile([P, NK, 4 * P], f16)       # 8KB/part
            wv_sb = ph1a.tile([P, NK, EC], f16)           # 4KB/part
            xT_r = xT.rearrange("(k p) s -> p k s", p=P)
            wqk_r = wqkT.rearrange("(k p) e -> p k e", p=P)
            wv_r = wvT.rearrange("(k p) e -> p k e", p=P)
            wo_r = woT.rearrange("(k p) e -> p k e", p=P)
            # prefetch on the sync queue; x k0 in 512-col chunks so the
            # first LN-stats matmul starts ~0.5us in, weights right after
            # so the first QKV Ldweights never stalls.
            for c in range(4):
                nc.sync.dma_start(out=xT_sb[:, 0, c * 512:(c + 1) * 512],
                                  in_=xT_r[:, 0, c * 512:(c + 1) * 512])
            nc.sync.dma_start(out=wv_sb[:], in_=wv_r[:])
            for k in range(1, NK):
                nc.sync.dma_start(out=xT_sb[:, k, :], in_=xT_r[:, k, :])
            nc.sync.dma_start(out=wqk_sb[:], in_=wqk_r[:])
            nc.sync.dma_start(out=wo_sb[:], in_=wo_r[:])
            nc.sync.dma_start(out=cos_sb[:], in_=cosf[:])
            nc.sync.dma_start(out=sin_sb[:], in_=sinsg[:])
            nc.sync.dma_start(out=nws_sb[:], in_=nws_qk[:])
            nc.sync.dma_start(out=nwsv_sb[:], in_=nws_v[:])

            # ---------------- phase 0: LN stats ----------------
            with tc.tile_pool(name="p0scr", bufs=1) as p0scr:
                rstd_sb = p0scr.tile([1, S], fp32)
                with tc.tile_pool(name="p0ps_a", bufs=1,
                                  space="PSUM") as p0ps_a:
                    pss = [p0ps_a.tile([1, 512], fp32, tag=f"pss{c}",
                                       name=f"pss{c}") for c in range(4)]
                    for k in range(NK):
                        for c in range(4):
                            nc.tensor.matmul(
                                pss[c][:], ones_sb,
                                xT_sb[:, k, c * 512:(c + 1) * 512],
                                start=(k == 0), stop=(k == NK - 1),
                                skip_group_check=True)
                    for c in range(4):   # mu (f16) from PSUM on Act
                        nc.scalar.mul(out=mu16[:, c * 512:(c + 1) * 512],
                                      in_=pss[c][:], mul=1.0 / D)
                with tc.tile_pool(name="p0ps_b", bufs=1,
                                  space="PSUM") as p0ps_b, \
                     tc.tile_pool(name="p0tmp", bufs=3) as p0tmp:
                    psq = [p0ps_b.tile([1, 512], fp32, tag=f"psq{c}",
                                       name=f"psq{c}") for c in range(4)]
                    for k in range(NK):
                        for h2 in range(2):
                            xsq = p0tmp.tile([P, S // 2], f16, tag="xsq")
                            nc.vector.tensor_mul(
                                xsq[:],
                                xT_sb[:, k, h2 * 1024:(h2 + 1) * 1024],
                                xT_sb[:, k, h2 * 1024:(h2 + 1) * 1024])
                            for c in range(2):
                                ci = h2 * 2 + c
                                nc.tensor.matmul(
                                    psq[ci][:], ones_sb,
                                    xsq[:, c * 512:(c + 1) * 512],
                                    start=(k == 0), stop=(k == NK - 1),
                                    skip_group_check=True)
                    # var = ssq/D - mu^2 ; rstd = 1/sqrt(var + eps)
                    nc.vector.tensor_mul(rstd_sb[:], mu16[:], mu16[:])
                    for c in range(4):
                        nc.vector.scalar_tensor_tensor(
                            out=rstd_sb[:, c * 512:(c + 1) * 512],
                            in0=psq[c][:], scalar=1.0 / D,
                            in1=rstd_sb[:, c * 512:(c + 1) * 512],
                            op0=OP.mult, op1=OP.subtract)
                    nc.scalar.activation(rstd_sb[:], rstd_sb[:], AF.Sqrt,
                                         bias=eps_sb)
                    nc.vector.reciprocal(out=rstd_sb[:], in_=rstd_sb[:])
                    nc.scalar.mul(out=rstd16[:], in_=rstd_sb[:], mul=1.0)

                    # partition-broadcast via DRAM bounce
                    rstd_d = dram_scr.tile([1, S], fp32)
                    rstd16_d = dram_scr.tile([1, S], f16)
                    nc.sync.dma_start(out=rstd_d[:], in_=rstd_sb[:])
                    nc.sync.dma_start(out=rstd16_d[:], in_=rstd16[:])
                    _rd, _r6 = rstd_d[:], rstd16_d[:]
                    nc.sync.dma_start(
                        out=rstd16_b[:],
                        in_=bass.AP(tensor=_r6.tensor, offset=_r6.offset,
                                    ap=[[0, P], [1, S]]))
                    # transposed per-s-tile scalars: [p, t] = vec[t*128 + p]
                    nc.sync.dma_start(
                        out=rstdT[:],
                        in_=bass.AP(tensor=_rd.tensor, offset=_rd.offset,
                                    ap=[[1, P], [P, NS]]))
                    nc.vector.tensor_scalar_mul(rstdT8[:], rstdT[:],
                                                float(HDIM) ** -0.5)

            # attention-wide pools (ps_s: 4 PSUM banks, shared by the
            # eager first blocks and the main loop; p2p holds ~2 blocks
            # of prob tiles for the 1-block-deep pipeline)
            blocks = [(h, ih) for h in range(HPC) for ih in range(2)]
            p_tiles = {b: [None] * NS for b in range(8)}
            with tc.tile_pool(name="ps_s", bufs=2, space="PSUM") as ps_s, \
                 tc.tile_pool(name="p2p", bufs=34) as p2p, \
                 tc.tile_pool(name="p2rec", bufs=2) as p2rec, \
                 tc.tile_pool(name="p2recd", bufs=2, space="DRAM") as p2recd:

                def emit_sc(b, j):
                    h, ih = blocks[b]
                    et, ep, i0 = h // 2, (h % 2) * HDIM, ih * SH
                    pscore = ps_s.tile([P, SH], fp32, tag="ps",
                                       name="pscore")
                    for c in range(2):
                        c0 = c * 512
                        nc.tensor.matmul(
                            pscore[:, c0:c0 + 512],
                            qk_sb[ep:ep + HDIM, 2 + et, j * P:(j + 1) * P],
                            qk_sb[ep:ep + HDIM, et, i0 + c0:i0 + c0 + 512],
                            start=True, stop=True, skip_group_check=True)
                    p_sb = p2p.tile([P, SH], f16, tag="p", name="p_sb")
                    nc.scalar.activation(p_sb[:], pscore[:], AF.Exp,
                                         scale=rstdT8[:, j:j + 1])
                    p_tiles[b][j] = p_sb

                def emit_av(b, j, po):
                    h, ih = blocks[b]
                    for c in range(2):
                        c0 = c * 512
                        nc.tensor.matmul(
                            po[:, c0:c0 + 512],
                            v_sb[:, j, h * VW:(h + 1) * VW],
                            p_tiles[b][j][:, c0:c0 + 512],
                            start=(j == 0), stop=(j == NS - 1),
                            skip_group_check=True)
                    if j == NS - 1:
                        p_tiles[b] = [None] * NS   # drop refs

                # ---------------- phase 1b first: V (no RoPE dep) -------
                with tc.tile_pool(name="p1vps", bufs=3,
                                  space="PSUM") as p1vps:
                    for t in range(NS):
                        zv = p1vps.tile([P, EC], fp32, tag="zv")
                        for k in range(NK):
                            nc.tensor.matmul(zv[:],
                                             xT_sb[:, k, t * P:(t + 1) * P],
                                             wv_sb[:, k, :],
                                             start=(k == 0), stop=False)
                        nc.tensor.matmul(zv[:], mu16[:, t * P:(t + 1) * P],
                                         nwsv_sb[:], start=False, stop=True)
                        # v = rstd_s * (Zv - mu*wsum)
                        nc.vector.tensor_scalar_mul(
                            v_sb[:, t, :].rearrange("p (h w) -> p h w",
                                                    h=HPC)[:, :, 0:HDIM],
                            zv[:].rearrange("p (h d) -> p h d", h=HPC),
                            rstdT[:, t:t + 1])

                # ---------------- phase 1a: Q/K matmuls + RoPE ----------
                with tc.tile_pool(name="p1psum", bufs=2,
                                  space="PSUM") as p1psum, \
                     tc.tile_pool(name="p1tmp", bufs=3) as p1tmp:
                    zq_cur = {}

                    def e_start(e, sh):
                        zq_cur[(e, sh)] = p1psum.tile([P, SH], fp32,
                                                      tag="zqk", name="zq")

                    def e_mm(e, sh, c, k):
                        s0, c0 = sh * SH, c * 512
                        nc.tensor.matmul(
                            zq_cur[(e, sh)][:, c0:c0 + 512],
                            wqk_sb[:, k, e * P:(e + 1) * P],
                            xT_sb[:, k, s0 + c0:s0 + c0 + 512],
                            start=(k == 0), stop=False,
                            skip_group_check=True)

                    def e_mu(e, sh, c):
                        s0, c0 = sh * SH, c * 512
                        nc.tensor.matmul(
                            zq_cur[(e, sh)][:, c0:c0 + 512],
                            nws_sb[:, e * P:(e + 1) * P],
                            mu16[:, s0 + c0:s0 + c0 + 512],
                            start=False, stop=True, skip_group_check=True)

                    def e_chain(e, sh):
                        s0 = sh * SH
                        zq = zq_cur.pop((e, sh))
                        u = p1tmp.tile([P, SH], f16, tag="u", name="u")
                        nc.vector.tensor_copy(out=u[:], in_=zq[:])
                        usw = p1tmp.tile([P, SH], f16, tag="usw",
                                         name="usw")
                        for g in range(2):   # rotate-half swap per 64-group
                            b0 = g * HDIM
                            nc.sync.dma_start(out=usw[b0:b0 + HALF, :],
                                              in_=u[b0 + HALF:b0 + HDIM, :])
                            nc.sync.dma_start(
                                out=usw[b0 + HALF:b0 + HDIM, :],
                                in_=u[b0:b0 + HALF, :])
                        nc.vector.tensor_mul(u[:], u[:],
                                             cos_sb[:, s0:s0 + SH])
                        nc.vector.tensor_mul(usw[:], usw[:],
                                             sin_sb[:, s0:s0 + SH])
                        if e < 2:   # q side: multiply by rstd as well
                            nc.vector.tensor_add(u[:], u[:], usw[:])
                            nc.vector.tensor_mul(qk_sb[:, e, s0:s0 + SH],
                                                 u[:],
                                                 rstd16_b[:, s0:s0 + SH])
                        else:
                            nc.vector.tensor_add(qk_sb[:, e, s0:s0 + SH],
                                                 u[:], usw[:])

                    def emit_group(e, sh):
                        e_start(e, sh)
                        for c in range(2):
                            for k in range(NK):
                                e_mm(e, sh, c, k)
                            e_mu(e, sh, c)
                        e_chain(e, sh)

                    # k0/q0 straight through
                    for sh in range(2):
                        emit_group(2, sh)
                    for sh in range(2):
                        emit_group(0, sh)
                    # k1/q1 interleaved with the first two blocks'
                    # scores+exp so ScalarE starts its 133us exp stream now
                    tasks = []
                    for e in (3, 1):
                        for sh in range(2):
                            tasks.append(("start", e, sh, 0, 0))
                            for c in range(2):
                                for k in range(NK):
                                    tasks.append(("mm", e, sh, c, k))
                                tasks.append(("mu", e, sh, c, 0))
                            tasks.append(("chain", e, sh, 0, 0))
                    ti = 0

                    def pop_tasks(n):
                        nonlocal ti
                        for _ in range(n):
                            if ti >= len(tasks):
                                return
                            kind, e, sh, c, k = tasks[ti]
                            ti += 1
                            if kind == "start":
                                e_start(e, sh)
                            elif kind == "mm":
                                e_mm(e, sh, c, k)
                            elif kind == "mu":
                                e_mu(e, sh, c)
                            else:
                                e_chain(e, sh)

                    for jj in range(2 * NS):
                        emit_sc(jj // NS, jj % NS)
                        pop_tasks(3)
                    pop_tasks(len(tasks))

                # ---------------- phase 2: attention main loop ----------
                with tc.tile_pool(name="ps_o", bufs=2,
                                  space="PSUM") as ps_o:
                    for b in range(8):
                        h, ih = blocks[b]
                        et, ep, i0 = h // 2, (h % 2) * HDIM, ih * SH
                        po = ps_o.tile([VW, SH], fp32, tag="po")
                        for j in range(NS):
                            if b + 2 < 8:
                                emit_sc(b + 2, j)
                            emit_av(b, j, po)
                        rec = p2rec.tile([1, SH], fp32, tag="rec")
                        nc.vector.reciprocal(out=rec[:],
                                             in_=po[HDIM:HDIM + 1, :])
                        rec_d = p2recd.tile([1, SH], fp32, tag="recd",
                                            name="rec_d")
                        nc.sync.dma_start(out=rec_d[:], in_=rec[:])
                        recb = p2rec.tile([HDIM, SH], fp32, tag="recb")
                        _rc = rec_d[:]
                        nc.sync.dma_start(
                            out=recb[:],
                            in_=bass.AP(tensor=_rc.tensor, offset=_rc.offset,
                                        ap=[[0, HDIM], [1, SH]]))
                        nc.vector.tensor_mul(
                            oT_sb[ep:ep + HDIM, et, i0:i0 + SH],
                            po[0:HDIM, :], recb[:])

                # ---------------- phase 3: out_proj partial -------------
                with tc.tile_pool(name="p3psum", bufs=2,
                                  space="PSUM") as p3psum, \
                     tc.tile_pool(name="p3tmp", bufs=3) as p3tmp:
                    for t in range(NS):
                        pout = p3psum.tile([P, D], fp32, tag="pout")
                        for c in range(2):   # fp32 PSUM: matmul N <= 512
                            c0 = c * 512
                            for k in range(2):
                                nc.tensor.matmul(
                                    pout[:, c0:c0 + 512],
                                    oT_sb[:, k, t * P:(t + 1) * P],
                                    wo_sb[:, k, c0:c0 + 512],
                                    start=(k == 0), stop=(k == 1),
                                    skip_group_check=True)
                        ot = p3tmp.tile([P, D], f16, tag="ot")
                        nc.scalar.mul(out=ot[:], in_=pout[:], mul=1.0)
                        nc.sync.dma_start(out=out[t * P:(t + 1) * P, :],
                                          in_=ot[:])

    nc.compile()
    return nc


def _host_inputs(x, ln_g, ln_b, w_qkv, w_out):
    f16 = np.float16
    wq = w_qkv[0:D] * ln_g[None, :]
    wk = w_qkv[D:2 * D] * ln_g[None, :]
    wv = w_qkv[2 * D:3 * D] * ln_g[None, :]
    if np.abs(w_qkv.astype(np.float32) @ ln_b.astype(np.float32)).max() != 0.0:
        raise NotImplementedError("nonzero ln_b not supported")
    inv = 1.0 / (ROPE_THETA ** (np.arange(0, HALF, dtype=np.float32) / HALF))
    fr = np.arange(S, dtype=np.float32)[:, None] * inv[None, :]
    cos = np.cos(fr).T.astype(np.float32)          # [32, S]
    sin = np.sin(fr).T.astype(np.float32)
    # row layout per 64-group: [lo(32); hi(32)]; cos same both halves.
    cosf = np.tile(cos, (4, 1)).astype(f16)        # [128, S]
    # rot_lo = lo*c - hi*s ; rot_hi = hi*c + lo*s. usw = [hi; lo], so the
    # sin multiplier rows are [-s (for lo out); +s (for hi out)].
    sinsg = np.tile(np.concatenate([-sin, sin], 0), (2, 1)).astype(f16)
    ins = []
    for core in range(N_CORES):
        b = core // 4
        h0 = (core % 4) * HPC
        sl = slice(h0 * HDIM, (h0 + HPC) * HDIM)
        wq_c, wk_c, wv_c = wq[sl], wk[sl], wv[sl]
        qk16 = np.concatenate([wq_c, wk_c], 0).astype(f16)
        wv16 = wv_c.astype(f16)
        # wsum from the f16-rounded weights so mu-folding matches; negated
        # because the K=1 matmul ACCUMULATES -mu*wsum into Z.
        nws_qk = (-qk16.astype(np.float32).sum(1)).astype(f16)[None, :]
        nws_v = (-wv16.astype(np.float32).sum(1)).astype(f16)[None, :]
        ins.append({
            "xT": np.ascontiguousarray(x[b].T.astype(f16)),
            "wqkT": np.ascontiguousarray(qk16.T),
            "wvT": np.ascontiguousarray(wv16.T),
            "woT": np.ascontiguousarray(w_out[:, sl].T.astype(f16)),
            "nws_qk": nws_qk, "nws_v": nws_v,
            "cosf": cosf, "sinsg": sinsg,
        })
    return ins


def kernel(x, ln_g, ln_b, w_qkv, w_out, b_out):
    from concourse import bass_utils
    x = np.asarray(x, np.float32)
    ln_g = np.asarray(ln_g, np.float32)
    ln_b = np.asarray(ln_b, np.float32)
    w_qkv = np.asarray(w_qkv, np.float32)
    w_out = np.asarray(w_out, np.float32)
    b_out = np.asarray(b_out, np.float32)
    if "nc" not in _cache:
        _cache["nc"] = _build()
    ins = _host_inputs(x, ln_g, ln_b, w_qkv, w_out)
    res = bass_utils.run_bass_kernel_spmd(_cache["nc"], ins,
                                          core_ids=list(range(N_CORES)))
    _cache["last_results"] = res
    out = np.zeros((B, S, D), np.float32)
    for core in range(N_CORES):
        out[core // 4] += res.results[core]["out"].astype(np.float32)
    out += b_out[None, None, :]
    return out
